# revision 19
# baseline (speedup 1.0000x reference)
"""AutoInt (nn_AutoInt_62156766707848) Trainium2 Bass kernel — v4.

Reference math (per sample b of B=2048):
    e   = emb_table[feat_index[b]]            # [F=64, D=128]
    q/k/v/r = e @ W{q,k,v,r}                  # [64, 512] each, split into H=8 heads of P=64
    s_h = q_h @ k_h^T                         # [64, 64]
    att = softmax(s, axis=q)                  # normalize over the QUERY axis
    av  = att @ v_h                           # [64, 64]
    multi = relu(concat_h(av) + e @ Wr)       # [64, 512]
    y   = sigmoid(multi.flatten() @ out_w + out_b)

Sharding: data-parallel over batch; 8 cores x 256 samples.

v4 design (vs v2/v3):
  - embeddings gathered + transposed on HOST; eT supertile slices streamed in
    by plain DMA (v3: killed the 16us/supertile GPSIMD dma_gather stall)
  - tail fused into ONE DVE scalar_tensor_tensor per mr bank:
    z_tok = sum_hp(relu(mr) * w2), accumulated into a zacc SBUF tile; single
    64KB output DMA at the end (kills GPS prod2, PE zz-MMs, ACT zsb copy)
  - pipeline deepened: iter st runs proj(st) | v-proj+vscale(st-1) |
    av-block+tail(st-2) | scores+exp+Z(st). The softmax chain
    (exp->Z->recip->vscale) has a full iteration of slack, so the PE never
    waits on it and HAM stays warm
  - engine balance: ACT = q/k copies + exp; DVE = Z-reduce + recip + vscale
    (4 wide ops) + fused tail; PE = matmuls only
  - av MMs ordered j-bank innermost, bb next: positions alternate every 2 MMs
    so LDWEIGHTS pulls ahead and the 2 diagonal tile positions overlap
"""

import sys

sys.path.insert(0, "/opt/trn_rl_repo")

from contextlib import ExitStack

import numpy as np
import ml_dtypes

import concourse.bass as bass
import concourse.tile as tile
from concourse import bacc, mybir
from concourse.bass_utils import run_bass_kernel_spmd

B, F, D, H, P, V = 2048, 64, 128, 8, 64, 100000
NCORES = 8
ST_SAMPLES = 8                # samples per supertile
TOK = ST_SAMPLES * F          # 512 tokens per supertile
PF = 3                        # eT prefetch depth

bf16 = mybir.dt.bfloat16
f32 = mybir.dt.float32

Exp = mybir.ActivationFunctionType.Exp
X = mybir.AxisListType.X
MUL = mybir.AluOpType.mult
MAX = mybir.AluOpType.max
ADD = mybir.AluOpType.add


def build_core_program(bc: int) -> bass.Bass:
    assert bc % ST_SAMPLES == 0
    nst = bc // ST_SAMPLES

    nc = bacc.Bacc("TRN2", target_bir_lowering=False, debug=False, num_devices=NCORES)

    et_d = nc.dram_tensor("et", [D, bc * F], bf16, kind="ExternalInput").ap()
    wq_d = nc.dram_tensor("wq", [D, H * P], bf16, kind="ExternalInput").ap()
    wk_d = nc.dram_tensor("wk", [D, H * P], bf16, kind="ExternalInput").ap()
    wv_d = nc.dram_tensor("wv", [D, H * P], bf16, kind="ExternalInput").ap()
    wr_d = nc.dram_tensor("wr", [D, H * P], bf16, kind="ExternalInput").ap()
    w2r_d = nc.dram_tensor("w2r", [128, H * P], bf16, kind="ExternalInput").ap()
    # z[tok_row, st*4+j]: per-token partial sums; host reduces 64 feats/sample
    zout = nc.dram_tensor("z", [128, nst * 4], f32, kind="ExternalOutput").ap()

    with tile.TileContext(nc) as tc:
        with ExitStack() as ctx:
            _body(ctx, tc, nst, et_d, (wq_d, wk_d, wv_d, wr_d), w2r_d, zout)
    nc.compile()
    return nc


def _body(ctx, tc, nst, et_d, w_drams, w2r_d, zout):
    nc = tc.nc

    cpool = ctx.enter_context(tc.tile_pool(name="const", bufs=1))
    egpool = ctx.enter_context(tc.tile_pool(name="eg", bufs=PF + 4))
    qkpool = ctx.enter_context(tc.tile_pool(name="qk", bufs=3))
    apool = ctx.enter_context(tc.tile_pool(name="att", bufs=12))
    vpool = ctx.enter_context(tc.tile_pool(name="vs", bufs=10))
    zpool = ctx.enter_context(tc.tile_pool(name="zr", bufs=3))
    mpool = ctx.enter_context(tc.tile_pool(name="m", bufs=2))

    zhpool = ctx.enter_context(tc.tile_pool(name="zh", bufs=4))

    # PSUM: 8 banks (pq 4 + sc 2 + mr 2)
    pq = ctx.enter_context(tc.tile_pool(name="pq", bufs=4, space="PSUM"))
    psc = ctx.enter_context(tc.tile_pool(name="psc", bufs=2, space="PSUM"))
    pmr = ctx.enter_context(tc.tile_pool(name="pmr", bufs=2, space="PSUM"))

    # ---- constants
    w_sb = []
    for name, wd in zip(("wq", "wk", "wv", "wr"), w_drams):
        t = cpool.tile([D, H * P], bf16, tag=name + "s", name=name + "s")
        nc.sync.dma_start(out=t[:], in_=wd[:, :])
        w_sb.append(t)
    wq_s, wk_s, wv_s, wr_s = w_sb

    w2r_s = cpool.tile([128, H * P], bf16, tag="w2rs")
    nc.sync.dma_start(out=w2r_s[:], in_=w2r_d[:, :])

    zacc = cpool.tile([128, nst * 4], f32, tag="zacc")

    def issue_load(g):
        eg = egpool.tile([128, TOK], bf16, tag="eg", name="eg")
        nc.sync.dma_start(out=eg[:], in_=et_d[:, g * TOK:(g + 1) * TOK])
        return eg

    eg_tiles = {g: issue_load(g) for g in range(min(PF, nst))}
    atts = {}    # st -> {(cp,hh): att tile [128=(bb,k), 512=(cin,j,q)]}
    zalls = {}   # st -> zall [128=(bb,k), 32=(j,cp,cin,hh)]

    def emit_A(st):
        """q/k projections (PE) + ACT copies -> qT/kT bf16 SBUF."""
        eT = eg_tiles[st][:]
        qT, kT = [None] * 4, [None] * 4
        for c in range(4):
            for w_s, lst, tag in ((wq_s, qT, "qT"), (wk_s, kT, "kT")):
                ps = pq.tile([128, TOK], f32, tag="proj", name="proj")
                nc.tensor.matmul(out=ps[:], lhsT=w_s[:, c * 128:(c + 1) * 128],
                                 rhs=eT, start=True, stop=True)
                t = qkpool.tile([128, TOK], bf16, tag=tag, name=tag)
                nc.scalar.copy(t[:], ps[:])
                lst[c] = t
        return qT, kT

    def emit_C(s):
        """av-block(s): recip + v-proj/scale + r + av accumulation + fused
        relu*w2 reduce tail. v and r share the eT-chunk stationary."""
        att_t, eT = atts.pop(s), eg_tiles.pop(s)[:]
        zall = zalls.pop(s)
        zr = zpool.tile([128, 32], f32, tag="Zr")
        nc.vector.reciprocal(zr[:, :], zall[:])
        vs = {}
        for ph in range(2):
            mr = {}
            for j in (2 * ph, 2 * ph + 1):
                # v and r back-to-back: same eT-chunk stationary operand
                ps = pq.tile([128, TOK], f32, tag="proj", name="vproj")
                nc.tensor.matmul(out=ps[:], lhsT=eT[:, j * 128:(j + 1) * 128],
                                 rhs=wv_s[:], start=True, stop=True)
                mr[j] = pmr.tile([128, TOK], f32, tag="mr", name=f"mr{j}")
                nc.tensor.matmul(out=mr[j][:],
                                 lhsT=eT[:, j * 128:(j + 1) * 128],
                                 rhs=wr_s[:], start=True, stop=False,
                                 skip_group_check=True)
                t = vpool.tile([128, TOK], bf16, tag="vs", name="vs")
                zrv = zr[:, j * 8:(j + 1) * 8].rearrange(
                    "p (h one) -> p h one", one=1).to_broadcast([128, 8, 64])
                nc.vector.tensor_tensor(
                    out=t[:].rearrange("p (h pp) -> p h pp", h=8),
                    in0=ps[:].rearrange("p (h pp) -> p h pp", h=8),
                    in1=zrv, op=MUL)
                vs[j] = t
            for cp in range(2):
                for cin in range(2):
                    c = 2 * cp + cin
                    for hh in range(2):
                        for bb in range(2):
                            for j in (2 * ph, 2 * ph + 1):
                                nc.tensor.matmul(
                                    out=mr[j][bb * 64:(bb + 1) * 64,
                                              (2 * c + hh) * 64:(2 * c + hh + 1) * 64],
                                    lhsT=att_t[(cp, hh)][bb * 64:(bb + 1) * 64,
                                                         (cin * 4 + j) * 64:(cin * 4 + j + 1) * 64],
                                    rhs=vs[j][bb * 64:(bb + 1) * 64,
                                              (2 * c + hh) * 64:(2 * c + hh + 1) * 64],
                                    start=False, stop=True,
                                    tile_position=(bb * 64, bb * 64),
                                    skip_group_check=True,
                                )
            for j in (2 * ph, 2 * ph + 1):
                dummy = mpool.tile([128, TOK], bf16, tag="p2d", name="p2d")
                col = s * 4 + j
                nc.vector.scalar_tensor_tensor(
                    out=dummy[:], in0=mr[j][:], scalar=0.0, in1=w2r_s[:],
                    op0=MAX, op1=MUL,
                    accum_out=zacc[:, col:col + 1])

    def emit_D(st, qT, kT):
        """scores(st) + exp + Z-reduce. zall cols = (j, cp, cin, hh)."""
        att_t = {}
        zall = zpool.tile([128, 32], f32, tag="Z")
        zv = zall[:].rearrange("p (j cp cin hh) -> p cin j cp hh",
                               j=4, cp=2, cin=2, hh=2)
        for cp in range(2):
            banks = [psc.tile([128, TOK], f32, tag="sc", name=f"sc{hh}")
                     for hh in range(2)]
            for cin in range(2):
                c = 2 * cp + cin
                for j in range(4):
                    for bb in range(2):
                        b = 2 * j + bb
                        for hh in range(2):
                            ro = hh * 64
                            nc.tensor.matmul(
                                out=banks[hh][bb * 64:(bb + 1) * 64,
                                              (cin * 4 + j) * 64:(cin * 4 + j + 1) * 64],
                                lhsT=kT[c][ro:ro + 64, b * 64:(b + 1) * 64],
                                rhs=qT[c][ro:ro + 64, b * 64:(b + 1) * 64],
                                start=True, stop=True,
                                tile_position=(ro, bb * 64),
                                skip_group_check=True,
                            )
            for hh in range(2):
                at = apool.tile([128, TOK], bf16, tag="att", name="att")
                nc.scalar.activation(out=at[:], in_=banks[hh][:], func=Exp)
                # Z = sum_q exp: two GPSIMD halving adds, DVE reduces the rest
                atv = at[:].rearrange("p (g q) -> p g q", q=64)
                th = zhpool.tile([128, 256], f32, tag="zh", name="zh")
                thv = th[:].rearrange("p (g i) -> p g i", i=32)
                nc.gpsimd.tensor_tensor(out=thv, in0=atv[:, :, 0:32],
                                        in1=atv[:, :, 32:64], op=ADD)
                t2 = zhpool.tile([128, 128], f32, tag="zh2", name="zh2")
                t2v = t2[:].rearrange("p (g i) -> p g i", i=16)
                nc.gpsimd.tensor_tensor(out=t2v, in0=thv[:, :, 0:16],
                                        in1=thv[:, :, 16:32], op=ADD)
                nc.vector.tensor_reduce(
                    out=zv[:, :, :, cp:cp + 1, hh:hh + 1],
                    in_=t2[:].rearrange("p (cin j i) -> p cin j i", cin=2, j=4),
                    axis=X, op=ADD)
                att_t[(cp, hh)] = at
        atts[st] = att_t
        zalls[st] = zall

    # Iteration order [C(st-2), D(st), A(st+1)]: scores consume q/k copied a
    # full iteration earlier, and the qk projection block runs stall-free at
    # iteration end (big MMs stay contiguous -> no tiling-mode-switch drains).
    qks = {0: emit_A(0)}
    for st in range(nst):
        if st + PF < nst and (st + PF) not in eg_tiles:
            eg_tiles[st + PF] = issue_load(st + PF)
        if st >= 2:
            emit_C(st - 2)
        emit_D(st, *qks.pop(st))
        if st + 1 < nst:
            qks[st + 1] = emit_A(st + 1)

    # epilogue: drain the pipeline
    emit_C(nst - 2)
    emit_C(nst - 1)
    nc.sync.dma_start(out=zout[:, :], in_=zacc[:])


_NC_CACHE: dict[int, bass.Bass] = {}


def _get_nc(bc: int) -> bass.Bass:
    if bc not in _NC_CACHE:
        _NC_CACHE[bc] = build_core_program(bc)
    return _NC_CACHE[bc]


def core_et(tokens: np.ndarray, emb_bf16: np.ndarray):
    """Host-side gather + transpose: eT [D, bc*F] bf16, column t = row token[t]."""
    return np.ascontiguousarray(emb_bf16[tokens].T)


def run_full(feat_index, emb_table, Wq, Wk, Wv, Wr, out_w, out_b, **spmd_kwargs):
    """Shard, run on 8 cores, unshard. Returns (y [B,1] f32, BassKernelResults)."""
    feat_index = np.asarray(feat_index)
    nb = feat_index.shape[0]
    bc = nb // NCORES
    nst = bc // ST_SAMPLES
    emb = np.asarray(emb_table, np.float32).astype(ml_dtypes.bfloat16)
    cores = [core_et(feat_index.reshape(NCORES, bc * F)[i], emb)
             for i in range(NCORES)]
    wq = np.asarray(Wq, np.float32).astype(ml_dtypes.bfloat16)
    wk = np.asarray(Wk, np.float32).astype(ml_dtypes.bfloat16)
    wv = np.asarray(Wv, np.float32).astype(ml_dtypes.bfloat16)
    wr = np.asarray(Wr, np.float32).astype(ml_dtypes.bfloat16)
    # w2rep [128, 512]: row (s*64 + f) = out_w.reshape(F, H*P)[f, :]
    w2 = np.asarray(out_w, np.float32).reshape(F, H * P)
    w2rep = np.concatenate([w2, w2], axis=0).astype(ml_dtypes.bfloat16)

    nc = _get_nc(bc)
    shared = {"wq": wq, "wk": wk, "wv": wv, "wr": wr, "w2r": w2rep}
    in_maps = [{"et": cores[i], **shared} for i in range(NCORES)]
    res = run_bass_kernel_spmd(nc, in_maps, core_ids=list(range(NCORES)), **spmd_kwargs)

    # z [128=(bb,k), nst*4=(st,j)] per core -> per-sample sums over k
    zs = []
    for r in res.results:
        z = r["z"].reshape(2, 64, nst, 4).sum(axis=1)     # [bb, st, j]
        zs.append(z.transpose(1, 2, 0).reshape(bc))       # sample = st*8+2j+bb
    z = np.concatenate(zs)
    z = z + np.float32(np.asarray(out_b, np.float32).reshape(-1)[0])
    y = 1.0 / (1.0 + np.exp(-z, dtype=np.float32))
    return y.reshape(nb, 1).astype(np.float32), res


def kernel(feat_index, emb_table, Wq, Wk, Wv, Wr, out_w, out_b):
    y, _ = run_full(feat_index, emb_table, Wq, Wk, Wv, Wr, out_w, out_b)
    return y
